# revision 35
# baseline (speedup 1.0000x reference)
"""Trainium2 Bass kernel for nn_MultiHeadAttention_39135742001649.

Reference computation (B=2, S=2048, D=1024, H=16, WIN=512):
    q/k/v = x @ W.T + b (per-head dk=64)
    scores = q k^T / 8                               [B,H,S,S]
    probs1 = blockwise softmax: causal mask, softmax within each 512-wide
             column block (masked entries -> 0)
    probs2 = full-row softmax(probs1)  (no masking; exp(0)=1 entries!)
    out    = (probs2 @ v) @ Wo.T + bo

Decomposition (validated vs reference):
    e1   = exp(scores) * tril_mask        (only 10 of 16 causal blocks)
    d1   = colsum of e1 within block      -> probs1 = e1 / d1
    e2   = exp(probs1)                    (masked/uncomputed entries -> 1)
    out_row = (sum_causal e2 @ v + suffix_colsum_v) / (sum_causal e2 + 512*(3-bi))

Sharding: 8 cores = 2 batches x 4 head-groups (4 heads each). Each core
computes q^T/k^T/v for its heads, the attention, and a partial output
projection over its 256 d-rows; the host sums the 4 partials per batch.

Structure: work proceeds in 4 "rounds", one per query block bi (ascending).
Round r's projection groups (q/k blocks, v chunks) are spread over the
round's first job slots, its attention jobs (h, bi=r, j<=r) stream through
a 4-stage skewed pipeline
  stage_a  (k):   scores matmuls -> [-1e30 triangle bias] -> exp1
  stage_b1a(k-2): d1 colsum matmuls -> reciprocal_approx_fast
  stage_b1b(k-3): s2 = e1*d1r (split DVE/GpSimd) -> exp2
  stage_b2 (k-5): PV(+d2 via ones columns) matmuls, fixup on last j
and the output projection for block r is interleaved as soon as its fixups
are issued. Diagonal jobs compute only live ranges (dead e2 regions memset
to 1.0 on GpSimd). Data path is bf16 (PSUM accumulation fp32); inputs are
host-laid-out in SBUF layout with big DMAs split across queues; 1/d via
DVE reciprocal_approx_fast (no activation-table swaps).
"""

import numpy as np
from contextlib import ExitStack

import concourse.bass as bass
import concourse.mybir as mybir
import concourse.tile as tile
from concourse import bacc
from concourse.bass_utils import run_bass_kernel_spmd

F32 = mybir.dt.float32
BF16 = mybir.dt.bfloat16
EXP = mybir.ActivationFunctionType.Exp
COPY = mybir.ActivationFunctionType.Copy
IDENT = mybir.ActivationFunctionType.Identity
ADD = mybir.AluOpType.add
MULT = mybir.AluOpType.mult
BYPASS = mybir.AluOpType.bypass

B, S, D, H, WIN = 2, 2048, 1024, 16, 512
DK = D // H          # 64
NB = S // WIN        # 4
NCORES = 8
HPC = 4              # heads per core
DCORE = HPC * DK     # 256
P = 128

TRACE = False        # set True from test.py to capture HW profile
TRACE_CORES = None

_CACHE = {}


def build_nc():
    nc = bacc.Bacc("TRN2", target_bir_lowering=False, debug=False)

    xT = nc.dram_tensor("xT", [P, NB, 8, WIN], BF16, kind="ExternalInput")
    wqT = nc.dram_tensor("wqT", [P, 8, DCORE], BF16, kind="ExternalInput")
    wkT = nc.dram_tensor("wkT", [P, 8, DCORE], BF16, kind="ExternalInput")
    wvT = nc.dram_tensor("wvT", [P, 8, DCORE], BF16, kind="ExternalInput")
    woT = nc.dram_tensor("woT", [P, 2, D], BF16, kind="ExternalInput")
    bq = nc.dram_tensor("bq", [P, 2], F32, kind="ExternalInput")         # /8
    bk = nc.dram_tensor("bk", [P, 2], F32, kind="ExternalInput")
    bvr = nc.dram_tensor("bvr", [P, DCORE], F32, kind="ExternalInput")   # bv replicated
    maskd = nc.dram_tensor("maskd", [P, P], F32, kind="ExternalInput")   # 0 / -1e30 bias
    sfxd = nc.dram_tensor("sfxd", [DK, 2, 2, NB], F32, kind="ExternalInput")
    outT = nc.dram_tensor("outT", [D, S], BF16, kind="ExternalOutput")   # partial out^T

    with tile.TileContext(nc) as tc, ExitStack() as ctx:
        const = ctx.enter_context(tc.tile_pool(name="const", bufs=1))
        wpool = ctx.enter_context(tc.tile_pool(name="wpool", bufs=1))
        persist = ctx.enter_context(tc.tile_pool(name="persist", bufs=1))

        mask_sb = const.tile([P, P], F32, name="mask_sb")
        nc.sync.dma_start(mask_sb[:], maskd[:])
        bq_sb = const.tile([P, 2], F32, name="bq_sb")
        nc.sync.dma_start(bq_sb[:], bq[:])
        bk_sb = const.tile([P, 2], F32, name="bk_sb")
        nc.sync.dma_start(bk_sb[:], bk[:])
        bvr_sb = const.tile([P, DCORE], F32, name="bvr_sb")
        nc.sync.dma_start(bvr_sb[:], bvr[:])
        sfx_sb = const.tile([DK, 2, 2, NB], F32, name="sfx_sb")  # suffix sums
        nc.sync.dma_start(sfx_sb[:], sfxd[:])

        ones128 = const.tile([P, P], BF16, name="ones128")
        nc.gpsimd.memset(ones128[:], 1.0)

        wq_sb = wpool.tile([P, 8, DCORE], BF16, name="wq_sb")
        wk_sb = wpool.tile([P, 8, DCORE], BF16, name="wk_sb")
        wv_sb = wpool.tile([P, 8, DCORE], BF16, name="wv_sb")
        wo_sb = wpool.tile([P, 2, D], BF16, name="wo_sb")


        qT_sb = persist.tile([P, 2, S], BF16, name="qT_sb")    # [d%128, d//128, s]
        kT_sb = persist.tile([P, 2, S], BF16, name="kT_sb")
        # Per head-pair padded V tiles for the PV matmul: even head's v in
        # cols 0:64 with ones in 64:128 (d2 lands in psum rows 64:128);
        # odd head's v in cols 64:128 with ones in 0:64 (d2 in rows 0:64).
        vE_sb = persist.tile([P, 16, 2, P], BF16, name="vE_sb")
        vO_sb = persist.tile([P, 16, 2, P], BF16, name="vO_sb")
        nc.gpsimd.memset(vE_sb[:, :, :, DK:P], 1.0)
        nc.gpsimd.memset(vO_sb[:, :, :, 0:DK], 1.0)
        attnT_sb = persist.tile([P, 2, S], BF16, name="attnT_sb")

        with (
            tc.tile_pool(name="xp", bufs=1) as xp,
            tc.tile_pool(name="e1p", bufs=5) as e1p,
            tc.tile_pool(name="s2p", bufs=2) as s2p,
            tc.tile_pool(name="e2p", bufs=4) as e2p,
            tc.tile_pool(name="drp", bufs=2) as drp,
            tc.tile_pool(name="drbp", bufs=3) as drbp,
            tc.tile_pool(name="epp", bufs=2) as epp,
            tc.tile_pool(name="d2sp", bufs=2) as d2sp,
            tc.tile_pool(name="otp", bufs=3) as otp,
            tc.tile_pool(name="psSC", bufs=4, space="PSUM") as psSC,
            tc.tile_pool(name="psD1", bufs=2, space="PSUM") as psD1,
            tc.tile_pool(name="psPV", bufs=2, space="PSUM") as psPV,
        ):
            x_sb = xp.tile([P, 8, S], BF16, name="x_sb")
            for st in range(NB):
                for part in range(4):
                    nc.sync.dma_start(
                        x_sb[:, 2 * part:2 * part + 2,
                             st * WIN:(st + 1) * WIN],
                        xT[:, st, 2 * part:2 * part + 2, :])
                    if st == 0:
                        nc.sync.dma_start(
                            wq_sb[:, 2 * part:2 * part + 2, :],
                            wqT[:, 2 * part:2 * part + 2, :])
                if st == 0:
                    for part in range(4):
                        nc.sync.dma_start(
                            wk_sb[:, 2 * part:2 * part + 2, :],
                            wkT[:, 2 * part:2 * part + 2, :])
                    for part in range(4):
                        nc.sync.dma_start(
                            wv_sb[:, 2 * part:2 * part + 2, :],
                            wvT[:, 2 * part:2 * part + 2, :])
                elif st == 1:
                    nc.sync.dma_start(wo_sb[:, 0, :], woT[:, 0, :])
                    nc.sync.dma_start(wo_sb[:, 1, :], woT[:, 1, :])

            def qk_proj(w_sb, b_sb, dst, st):
                for dc in range(2):
                    ps = psSC.tile([P, WIN], F32, name="sc_ps")
                    for o in range(8):
                        nc.tensor.matmul(ps[:],
                                         w_sb[:, o, dc * P:(dc + 1) * P],
                                         x_sb[:, o, st * WIN:(st + 1) * WIN],
                                         start=(o == 0), stop=(o == 7))
                    nc.vector.tensor_scalar_add(
                        dst[:, dc, st * WIN:(st + 1) * WIN], ps[:],
                        b_sb[:, dc:dc + 1])

            def v_proj(sc):
                ps = psD1.tile([P, WIN], F32, name="d1_ps")
                for o in range(8):
                    nc.tensor.matmul(ps[:, 0:DCORE],
                                     x_sb[:, o, sc * P:(sc + 1) * P],
                                     wv_sb[:, o, :],
                                     start=(o == 0), stop=(o == 7))
                for hc in range(2):
                    e0 = (2 * hc) * DK
                    o0 = (2 * hc + 1) * DK
                    nc.vector.tensor_tensor(vE_sb[:, sc, hc, 0:DK],
                                            ps[:, e0:e0 + DK],
                                            bvr_sb[:, e0:e0 + DK], ADD)
                    nc.vector.tensor_tensor(vO_sb[:, sc, hc, DK:P],
                                            ps[:, o0:o0 + DK],
                                            bvr_sb[:, o0:o0 + DK], ADD)

            def proj_round(r):
                qk_proj(wq_sb, bq_sb, qT_sb, r)
                qk_proj(wk_sb, bk_sb, kT_sb, r)
                for sc in range(4 * r, 4 * r + 4):
                    v_proj(sc)

            state = {}

            def stage_a(job):
                h, bi, j = job
                hc, hb = h // 2, (h % 2) * DK
                diag = (j == bi)
                e1 = e1p.tile([P, NB, WIN], BF16, name="e1")
                for m in range(NB):
                    lo = m * P if diag else 0
                    sc_ps = psSC.tile([P, WIN], F32, name="sc_ps")
                    lhsT = kT_sb[hb:hb + DK, hc,
                                 j * WIN + m * P: j * WIN + (m + 1) * P]
                    rhs = qT_sb[hb:hb + DK, hc,
                                bi * WIN + lo:(bi + 1) * WIN]
                    nc.tensor.matmul(sc_ps[:, lo:], lhsT, rhs,
                                     start=True, stop=True)
                    if diag:
                        # -1e30 bias on the in-chunk triangle: exp -> 0
                        nc.vector.tensor_tensor(
                            sc_ps[:, lo:lo + P],
                            sc_ps[:, lo:lo + P], mask_sb[:], ADD)
                    nc.scalar.activation(e1[:, m, lo:], sc_ps[:, lo:], EXP)
                state[job] = e1

            def stage_b1a(job):
                h, bi, j = job
                diag = (j == bi)
                e1 = state[job]
                d1_ps = psD1.tile([P, WIN], F32, name="d1_ps")
                for m in range(NB):
                    lo = m * P if diag else 0
                    nc.tensor.matmul(d1_ps[:, lo:], ones128[:], e1[:, m, lo:],
                                     start=(m == 0), stop=(m == 3))
                d1r = drp.tile([P, WIN], F32, name="d1r")
                nc.vector.reciprocal_approx_fast(d1r[:], d1_ps[:])
                state[(job, "dr")] = d1r

            def stage_b1b(job):
                h, bi, j = job
                diag = (j == bi)
                e1 = state.pop(job)
                d1r = state.pop((job, "dr"))
                s2 = s2p.tile([P, NB, WIN], BF16, name="s2")
                e2 = e2p.tile([P, NB, WIN], BF16, name="e2")
                for m in range(NB):
                    lo = m * P if diag else 0
                    eng = nc.vector if m < 2 else nc.gpsimd
                    eng.tensor_tensor(s2[:, m, lo:], e1[:, m, lo:],
                                      d1r[:, lo:], MULT)
                if diag:
                    for m in range(1, NB):
                        nc.gpsimd.memset(e2[:, m, 0:m * P], 1.0)
                    for m in range(NB):
                        lo = m * P
                        nc.scalar.activation(e2[:, m, lo:], s2[:, m, lo:], EXP)
                else:
                    nc.scalar.activation(e2[:], s2[:], EXP)
                state[job] = e2

            def stage_b2(job):
                h, bi, j = job
                hc, hb = h // 2, (h % 2) * DK
                vh = vE_sb if h % 2 == 0 else vO_sb
                e2 = state.pop(job)
                if j == 0:
                    state[(h, bi, "pv")] = psPV.tile([P, WIN], F32, name="pv_ps")
                pv_ps = state[(h, bi, "pv")]
                first = (j == 0)
                last = (j == bi)
                for m in range(NB):
                    nc.tensor.matmul(pv_ps[:, :], vh[:, j * 4 + m, hc, :],
                                     e2[:, m, :],
                                     start=(first and m == 0),
                                     stop=(last and m == 3))
                if not last:
                    return
                # fixup: attnT = (pv + sfx) / (d2 + 512*(3-bi))
                pv_ps = state.pop((h, bi, "pv"))
                d2s = d2sp.tile([P, WIN], F32, name="d2s")
                d2r = d2sp.tile([P, WIN], F32, name="d2r")
                cst = float(WIN * (NB - 1 - bi))
                opp = DK - hb  # d2 rows live at the opposite 64-row half
                nc.scalar.activation(d2s[0:DK, :], pv_ps[opp:opp + DK, :],
                                     COPY, bias=cst)
                nc.vector.reciprocal_approx_fast(d2r[0:DK, :], d2s[0:DK, :])
                nc.vector.scalar_tensor_tensor(
                    attnT_sb[hb:hb + DK, hc, bi * WIN:(bi + 1) * WIN],
                    pv_ps[hb:hb + DK, :],
                    sfx_sb[0:DK, hb // DK, hc, bi:bi + 1],
                    d2r[0:DK, :], ADD, MULT)

            def out_proj(st):
                # output projection for query block st; needs attnT[:, :, st]
                for ec in range(8):
                    ps = psPV.tile([P, WIN], F32, name="pv_ps")
                    for dsub in range(2):
                        nc.tensor.matmul(
                            ps[:], wo_sb[:, dsub, ec * P:(ec + 1) * P],
                            attnT_sb[:, dsub, st * WIN:(st + 1) * WIN],
                            start=(dsub == 0), stop=(dsub == 1))
                    ot = otp.tile([P, WIN], BF16, name="ot")
                    nc.vector.tensor_copy(ot[:], ps[:])
                    hw = WIN // 2
                    nc.sync.dma_start(
                        outT[ec * P:(ec + 1) * P,
                             st * WIN:st * WIN + hw], ot[:, 0:hw])
                    nc.sync.dma_start(
                        outT[ec * P:(ec + 1) * P,
                             st * WIN + hw:(st + 1) * WIN], ot[:, hw:])

            # rounds: proj parts for block r spread over the round's first
            # job slots (q gates scores; k only the diagonal job; v only PV,
            # which runs 5 slots later), jobs (h, r, j<=r), then out block r
            jobs = [(h, bi, j) for bi in range(NB) for h in range(HPC)
                    for j in range(bi + 1)]
            n = len(jobs)
            round_base = {0: 0, 1: 4, 2: 12, 3: 24}
            parts_at = {}
            for r in range(NB):
                base = round_base[r]
                qp = (lambda rr: lambda: qk_proj(wq_sb, bq_sb, qT_sb, rr))(r)
                kp = (lambda rr: lambda: qk_proj(wk_sb, bk_sb, kT_sb, rr))(r)
                parts_at.setdefault(base, []).append(qp)
                parts_at.setdefault(base + min(2, r), []).append(kp)
                for c in range(4):
                    vp = (lambda ss: lambda: v_proj(ss))(4 * r + c)
                    if r == 0:
                        pos = 2 + c
                    else:
                        # must be issued before PV of the diagonal job
                        # (h=0, r, j=r) at slot base + r + 5
                        pos = min(4 + 2 * c, r + 4)
                    parts_at.setdefault(base + pos, []).append(vp)
            # out_proj(st) is ready at k = last-fixup-idx + 6; issue it late
            # in the NEXT round, where the PE otherwise idles behind Scalar
            outp_fire = {10: 0, 20: 1, 34: 2, 44: 3}
            for k in range(n + 5):
                for part in parts_at.get(k, ()):
                    part()
                if k < n:
                    stage_a(jobs[k])
                if 0 <= k - 5 < n:
                    stage_b2(jobs[k - 5])
                if 0 <= k - 2 < n:
                    stage_b1a(jobs[k - 2])
                if 0 <= k - 3 < n:
                    stage_b1b(jobs[k - 3])
                if k in outp_fire:
                    out_proj(outp_fire[k])

    nc.compile()
    return nc


def make_in_maps(x, Wq_w, Wq_b, Wk_w, Wk_b, Wv_w, Wv_b, Wo_w, Wo_b):
    from ml_dtypes import bfloat16

    def bfc(a):
        return np.ascontiguousarray(np.asarray(a, np.float32).astype(bfloat16))

    x = np.asarray(x, np.float32)
    Wq8 = np.asarray(Wq_w, np.float32) / 8.0
    bq8 = np.asarray(Wq_b, np.float32) / 8.0
    wqT = bfc(Wq8.T)
    wkT = bfc(np.asarray(Wk_w, np.float32).T)
    wvT = bfc(np.asarray(Wv_w, np.float32).T)
    woT = bfc(np.asarray(Wo_w, np.float32).T)

    def w_lay(wt, dsl):      # [D, DCORE slice] -> [P, 8, DCORE]
        return np.ascontiguousarray(
            wt[:, dsl].reshape(8, P, DCORE).transpose(1, 0, 2))

    def b_lay(bv):           # [DCORE] -> [P, 2]
        return np.ascontiguousarray(np.asarray(bv, np.float32)
                                    .reshape(2, P).T)

    # additive mask for the in-chunk diagonal: 0 where k<=q else -1e30
    tri = (np.arange(P)[:, None] <= np.arange(P)[None, :])
    maskb = np.where(tri, 0.0, -1e30).astype(np.float32)

    xTb = [bfc(x[b].T) for b in range(B)]
    xh = [np.ascontiguousarray(
        t.reshape(8, P, NB, WIN).transpose(1, 2, 0, 3)) for t in xTb]

    in_maps = []
    for core in range(NCORES):
        b = core // 4
        h0 = (core % 4) * HPC
        dsl = slice(h0 * DK, (h0 + HPC) * DK)
        bv_core = np.asarray(Wv_b, np.float32)[dsl]
        # suffix colsum(v) table computed on host from the rounded operands:
        # colsum_j(v) = (sum_{s in block j} x[s,:]) @ WvT_core + 512*bv
        wvT_core = np.ascontiguousarray(wvT[:, dsl]).astype(np.float32)
        xb32 = xTb[b].astype(np.float32)
        rowsum = np.stack([xb32[:, j * WIN:(j + 1) * WIN].sum(axis=1)
                           for j in range(NB)])            # [NB, D]
        cs = rowsum @ wvT_core + WIN * bv_core[None, :]     # [NB, DCORE]
        sfx_full = np.zeros((NB, DCORE), np.float32)
        for bi in range(NB - 1):
            sfx_full[bi] = cs[bi + 1:].sum(axis=0)
        sfx = np.zeros((DK, 2, 2, NB), np.float32)
        for hc in range(2):
            for half in range(2):
                for bi in range(NB):
                    sfx[:, half, hc, bi] = sfx_full[bi][
                        hc * P + half * DK: hc * P + half * DK + DK]
        in_maps.append({
            "xT": xh[b],
            "wqT": w_lay(wqT, dsl),
            "wkT": w_lay(wkT, dsl),
            "wvT": w_lay(wvT, dsl),
            "woT": np.ascontiguousarray(
                woT[dsl, :].reshape(2, P, D).transpose(1, 0, 2)),
            "bq": b_lay(bq8[dsl]),
            "bk": b_lay(np.asarray(Wk_b, np.float32)[dsl]),
            "bvr": np.ascontiguousarray(np.broadcast_to(bv_core, (P, DCORE))),
            "maskd": maskb,
            "sfxd": sfx,
        })
    return in_maps


def kernel(**inputs):
    if "nc" not in _CACHE:
        _CACHE["nc"] = build_nc()
    nc = _CACHE["nc"]
    in_maps = make_in_maps(**inputs)
    kw = {}
    if TRACE:
        kw["trace"] = True
        if TRACE_CORES is not None:
            kw["trace_cores"] = TRACE_CORES
    res = run_bass_kernel_spmd(nc, in_maps, list(range(NCORES)), **kw)
    _CACHE["last_result"] = res

    bo = np.asarray(inputs["Wo_b"], np.float32)
    out = np.zeros((B, S, D), np.float32)
    for b in range(B):
        acc = np.zeros((D, S), np.float32)
        for core in range(b * 4, b * 4 + 4):
            acc += np.asarray(res.results[core]["outT"], np.float32)
        out[b] = acc.T + bo
    return out


# revision 36
# speedup vs baseline: 1.0360x; 1.0360x over previous
"""Trainium2 Bass kernel for nn_MultiHeadAttention_39135742001649.

Reference computation (B=2, S=2048, D=1024, H=16, WIN=512):
    q/k/v = x @ W.T + b (per-head dk=64)
    scores = q k^T / 8                               [B,H,S,S]
    probs1 = blockwise softmax: causal mask, softmax within each 512-wide
             column block (masked entries -> 0)
    probs2 = full-row softmax(probs1)  (no masking; exp(0)=1 entries!)
    out    = (probs2 @ v) @ Wo.T + bo

Decomposition (validated vs reference):
    e1   = exp(scores) * tril_mask        (only 10 of 16 causal blocks)
    d1   = colsum of e1 within block      -> probs1 = e1 / d1
    e2   = exp(probs1)                    (masked/uncomputed entries -> 1)
    out_row = (sum_causal e2 @ v + suffix_colsum_v) / (sum_causal e2 + 512*(3-bi))

Sharding: 8 cores = 2 batches x 4 head-groups (4 heads each). Each core
computes q^T/k^T/v for its heads, the attention, and a partial output
projection over its 256 d-rows; the host sums the 4 partials per batch.

Structure: work proceeds in 4 "rounds", one per query block bi (ascending).
Round r projects q(block r), k(block r), v(chunks 4r..4r+4), then runs all
attention jobs (h, bi=r, j<=r) through a 4-stage skewed pipeline
(scores+exp1 | d1+recip | s2+exp2 | PV+fixup), then the output projection
for block r.  This keeps the in-order PE queue fed from ~20us onward.

Data path is bf16 (PSUM accumulation fp32). 1/d via DVE
reciprocal_approx_fast (no activation-table swaps).
"""

import numpy as np
from contextlib import ExitStack

import concourse.bass as bass
import concourse.mybir as mybir
import concourse.tile as tile
from concourse import bacc
from concourse.bass_utils import run_bass_kernel_spmd

F32 = mybir.dt.float32
BF16 = mybir.dt.bfloat16
EXP = mybir.ActivationFunctionType.Exp
COPY = mybir.ActivationFunctionType.Copy
IDENT = mybir.ActivationFunctionType.Identity
ADD = mybir.AluOpType.add
MULT = mybir.AluOpType.mult
BYPASS = mybir.AluOpType.bypass

B, S, D, H, WIN = 2, 2048, 1024, 16, 512
DK = D // H          # 64
NB = S // WIN        # 4
NCORES = 8
HPC = 4              # heads per core
DCORE = HPC * DK     # 256
P = 128

TRACE = False        # set True from test.py to capture HW profile
TRACE_CORES = None

_CACHE = {}


def build_nc():
    nc = bacc.Bacc("TRN2", target_bir_lowering=False, debug=False)

    xT = nc.dram_tensor("xT", [P, NB, 8, WIN], BF16, kind="ExternalInput")
    wqT = nc.dram_tensor("wqT", [P, 8, DCORE], BF16, kind="ExternalInput")
    wkT = nc.dram_tensor("wkT", [P, 8, DCORE], BF16, kind="ExternalInput")
    wvT = nc.dram_tensor("wvT", [P, 8, DCORE], BF16, kind="ExternalInput")
    woT = nc.dram_tensor("woT", [P, 2, D], BF16, kind="ExternalInput")
    bq = nc.dram_tensor("bq", [P, 2], F32, kind="ExternalInput")         # /8
    bk = nc.dram_tensor("bk", [P, 2], F32, kind="ExternalInput")
    bvr = nc.dram_tensor("bvr", [P, DCORE], F32, kind="ExternalInput")   # bv replicated
    maskd = nc.dram_tensor("maskd", [P, P], F32, kind="ExternalInput")   # 0 / -1e30 bias
    sfxd = nc.dram_tensor("sfxd", [DK, 2, 2, NB], F32, kind="ExternalInput")
    outT = nc.dram_tensor("outT", [D, S], BF16, kind="ExternalOutput")   # partial out^T

    with tile.TileContext(nc) as tc, ExitStack() as ctx:
        const = ctx.enter_context(tc.tile_pool(name="const", bufs=1))
        wpool = ctx.enter_context(tc.tile_pool(name="wpool", bufs=1))
        persist = ctx.enter_context(tc.tile_pool(name="persist", bufs=1))

        mask_sb = const.tile([P, P], F32, name="mask_sb")
        nc.sync.dma_start(mask_sb[:], maskd[:])
        bq_sb = const.tile([P, 2], F32, name="bq_sb")
        nc.sync.dma_start(bq_sb[:], bq[:])
        bk_sb = const.tile([P, 2], F32, name="bk_sb")
        nc.sync.dma_start(bk_sb[:], bk[:])
        bvr_sb = const.tile([P, DCORE], F32, name="bvr_sb")
        nc.sync.dma_start(bvr_sb[:], bvr[:])
        sfx_sb = const.tile([DK, 2, 2, NB], F32, name="sfx_sb")  # suffix sums
        nc.sync.dma_start(sfx_sb[:], sfxd[:])

        ones128 = const.tile([P, P], BF16, name="ones128")
        nc.gpsimd.memset(ones128[:], 1.0)

        wq_sb = wpool.tile([P, 8, DCORE], BF16, name="wq_sb")
        wk_sb = wpool.tile([P, 8, DCORE], BF16, name="wk_sb")
        wv_sb = wpool.tile([P, 8, DCORE], BF16, name="wv_sb")
        wo_sb = wpool.tile([P, 2, D], BF16, name="wo_sb")


        qT_sb = persist.tile([P, 2, S], BF16, name="qT_sb")    # [d%128, d//128, s]
        kT_sb = persist.tile([P, 2, S], BF16, name="kT_sb")
        # Per head-pair padded V tiles for the PV matmul: even head's v in
        # cols 0:64 with ones in 64:128 (d2 lands in psum rows 64:128);
        # odd head's v in cols 64:128 with ones in 0:64 (d2 in rows 0:64).
        vE_sb = persist.tile([P, 16, 2, P], BF16, name="vE_sb")
        vO_sb = persist.tile([P, 16, 2, P], BF16, name="vO_sb")
        nc.gpsimd.memset(vE_sb[:, :, :, DK:P], 1.0)
        nc.gpsimd.memset(vO_sb[:, :, :, 0:DK], 1.0)
        attnT_sb = persist.tile([P, 2, S], BF16, name="attnT_sb")

        with (
            tc.tile_pool(name="xp", bufs=1) as xp,
            tc.tile_pool(name="e1p", bufs=5) as e1p,
            tc.tile_pool(name="s2p", bufs=2) as s2p,
            tc.tile_pool(name="e2p", bufs=4) as e2p,
            tc.tile_pool(name="drp", bufs=2) as drp,
            tc.tile_pool(name="drbp", bufs=3) as drbp,
            tc.tile_pool(name="epp", bufs=2) as epp,
            tc.tile_pool(name="d2sp", bufs=2) as d2sp,
            tc.tile_pool(name="otp", bufs=3) as otp,
            tc.tile_pool(name="psSC", bufs=4, space="PSUM") as psSC,
            tc.tile_pool(name="psD1", bufs=2, space="PSUM") as psD1,
            tc.tile_pool(name="psPV", bufs=2, space="PSUM") as psPV,
        ):
            x_sb = xp.tile([P, 8, S], BF16, name="x_sb")
            for st in range(NB):
                for part in range(4):
                    nc.sync.dma_start(
                        x_sb[:, 2 * part:2 * part + 2,
                             st * WIN:(st + 1) * WIN],
                        xT[:, st, 2 * part:2 * part + 2, :])
                    if st == 0:
                        nc.sync.dma_start(
                            wq_sb[:, 2 * part:2 * part + 2, :],
                            wqT[:, 2 * part:2 * part + 2, :])
                if st == 0:
                    for part in range(4):
                        nc.sync.dma_start(
                            wk_sb[:, 2 * part:2 * part + 2, :],
                            wkT[:, 2 * part:2 * part + 2, :])
                    for part in range(4):
                        nc.sync.dma_start(
                            wv_sb[:, 2 * part:2 * part + 2, :],
                            wvT[:, 2 * part:2 * part + 2, :])
                elif st == 1:
                    nc.sync.dma_start(wo_sb[:, 0, :], woT[:, 0, :])
                    nc.sync.dma_start(wo_sb[:, 1, :], woT[:, 1, :])

            def qk_proj(w_sb, b_sb, dst, st):
                for dc in range(2):
                    ps = psSC.tile([P, WIN], F32, name="sc_ps")
                    for o in range(8):
                        nc.tensor.matmul(ps[:],
                                         w_sb[:, o, dc * P:(dc + 1) * P],
                                         x_sb[:, o, st * WIN:(st + 1) * WIN],
                                         start=(o == 0), stop=(o == 7))
                    nc.vector.tensor_scalar_add(
                        dst[:, dc, st * WIN:(st + 1) * WIN], ps[:],
                        b_sb[:, dc:dc + 1])

            def v_proj(sc):
                ps = psD1.tile([P, WIN], F32, name="d1_ps")
                for o in range(8):
                    nc.tensor.matmul(ps[:, 0:DCORE],
                                     x_sb[:, o, sc * P:(sc + 1) * P],
                                     wv_sb[:, o, :],
                                     start=(o == 0), stop=(o == 7))
                for hc in range(2):
                    e0 = (2 * hc) * DK
                    o0 = (2 * hc + 1) * DK
                    nc.vector.tensor_tensor(vE_sb[:, sc, hc, 0:DK],
                                            ps[:, e0:e0 + DK],
                                            bvr_sb[:, e0:e0 + DK], ADD)
                    nc.vector.tensor_tensor(vO_sb[:, sc, hc, DK:P],
                                            ps[:, o0:o0 + DK],
                                            bvr_sb[:, o0:o0 + DK], ADD)

            def proj_round(r):
                qk_proj(wq_sb, bq_sb, qT_sb, r)
                qk_proj(wk_sb, bk_sb, kT_sb, r)
                for sc in range(4 * r, 4 * r + 4):
                    v_proj(sc)

            state = {}

            def stage_a(job):
                h, bi, j = job
                hc, hb = h // 2, (h % 2) * DK
                diag = (j == bi)
                e1 = e1p.tile([P, NB, WIN], BF16, name="e1")
                for m in range(NB):
                    lo = m * P if diag else 0
                    sc_ps = psSC.tile([P, WIN], F32, name="sc_ps")
                    lhsT = kT_sb[hb:hb + DK, hc,
                                 j * WIN + m * P: j * WIN + (m + 1) * P]
                    rhs = qT_sb[hb:hb + DK, hc,
                                bi * WIN + lo:(bi + 1) * WIN]
                    nc.tensor.matmul(sc_ps[:, lo:], lhsT, rhs,
                                     start=True, stop=True)
                    if diag:
                        # -1e30 bias on the in-chunk triangle: exp -> 0
                        nc.vector.tensor_tensor(
                            sc_ps[:, lo:lo + P],
                            sc_ps[:, lo:lo + P], mask_sb[:], ADD)
                    nc.scalar.activation(e1[:, m, lo:], sc_ps[:, lo:], EXP)
                state[job] = e1

            def stage_b1a(job):
                h, bi, j = job
                diag = (j == bi)
                e1 = state[job]
                d1_ps = psD1.tile([P, WIN], F32, name="d1_ps")
                for m in range(NB):
                    lo = m * P if diag else 0
                    nc.tensor.matmul(d1_ps[:, lo:], ones128[:], e1[:, m, lo:],
                                     start=(m == 0), stop=(m == 3))
                d1r = drp.tile([P, WIN], F32, name="d1r")
                nc.vector.reciprocal_approx_fast(d1r[:], d1_ps[:])
                state[(job, "dr")] = d1r

            def stage_b1b(job):
                h, bi, j = job
                diag = (j == bi)
                e1 = state.pop(job)
                d1r = state.pop((job, "dr"))
                s2 = s2p.tile([P, NB, WIN], BF16, name="s2")
                e2 = e2p.tile([P, NB, WIN], BF16, name="e2")
                for m in range(NB):
                    lo = m * P if diag else 0
                    eng = nc.vector if m < 2 else nc.gpsimd
                    eng.tensor_tensor(s2[:, m, lo:], e1[:, m, lo:],
                                      d1r[:, lo:], MULT)
                if diag:
                    for m in range(1, NB):
                        nc.gpsimd.memset(e2[:, m, 0:m * P], 1.0)
                    for m in range(NB):
                        lo = m * P
                        nc.scalar.activation(e2[:, m, lo:], s2[:, m, lo:], EXP)
                else:
                    nc.scalar.activation(e2[:], s2[:], EXP)
                state[job] = e2

            def stage_b2(job):
                h, bi, j = job
                hc, hb = h // 2, (h % 2) * DK
                vh = vE_sb if h % 2 == 0 else vO_sb
                e2 = state.pop(job)
                if j == 0:
                    state[(h, bi, "pv")] = psPV.tile([P, WIN], F32, name="pv_ps")
                pv_ps = state[(h, bi, "pv")]
                first = (j == 0)
                last = (j == bi)
                for m in range(NB):
                    nc.tensor.matmul(pv_ps[:, :], vh[:, j * 4 + m, hc, :],
                                     e2[:, m, :],
                                     start=(first and m == 0),
                                     stop=(last and m == 3))
                if not last:
                    return
                # fixup: attnT = (pv + sfx) / (d2 + 512*(3-bi))
                pv_ps = state.pop((h, bi, "pv"))
                d2s = d2sp.tile([P, WIN], F32, name="d2s")
                d2r = d2sp.tile([P, WIN], F32, name="d2r")
                cst = float(WIN * (NB - 1 - bi))
                opp = DK - hb  # d2 rows live at the opposite 64-row half
                nc.scalar.activation(d2s[0:DK, :], pv_ps[opp:opp + DK, :],
                                     COPY, bias=cst)
                nc.vector.reciprocal_approx_fast(d2r[0:DK, :], d2s[0:DK, :])
                nc.vector.scalar_tensor_tensor(
                    attnT_sb[hb:hb + DK, hc, bi * WIN:(bi + 1) * WIN],
                    pv_ps[hb:hb + DK, :],
                    sfx_sb[0:DK, hb // DK, hc, bi:bi + 1],
                    d2r[0:DK, :], ADD, MULT)

            def out_proj(st):
                # output projection for query block st; needs attnT[:, :, st]
                for ec in range(8):
                    ps = psPV.tile([P, WIN], F32, name="pv_ps")
                    for dsub in range(2):
                        nc.tensor.matmul(
                            ps[:], wo_sb[:, dsub, ec * P:(ec + 1) * P],
                            attnT_sb[:, dsub, st * WIN:(st + 1) * WIN],
                            start=(dsub == 0), stop=(dsub == 1))
                    ot = otp.tile([P, WIN], BF16, name="ot")
                    nc.vector.tensor_copy(ot[:], ps[:])
                    hw = WIN // 2
                    nc.sync.dma_start(
                        outT[ec * P:(ec + 1) * P,
                             st * WIN:st * WIN + hw], ot[:, 0:hw])
                    nc.sync.dma_start(
                        outT[ec * P:(ec + 1) * P,
                             st * WIN + hw:(st + 1) * WIN], ot[:, hw:])

            # rounds: proj parts for block r spread over the round's first
            # job slots (q gates scores; k only the diagonal job; v only PV,
            # which runs 5 slots later), jobs (h, r, j<=r), then out block r
            jobs = [(h, bi, j) for bi in range(NB) for h in range(HPC)
                    for j in range(bi + 1)]
            n = len(jobs)
            round_base = {0: 0, 1: 4, 2: 12, 3: 24}
            parts_at = {}
            for r in range(NB):
                base = round_base[r]
                qp = (lambda rr: lambda: qk_proj(wq_sb, bq_sb, qT_sb, rr))(r)
                kp = (lambda rr: lambda: qk_proj(wk_sb, bk_sb, kT_sb, rr))(r)
                parts_at.setdefault(base, []).append(qp)
                parts_at.setdefault(base + min(2, r), []).append(kp)
                for c in range(4):
                    vp = (lambda ss: lambda: v_proj(ss))(4 * r + c)
                    if r == 0:
                        pos = 2 + c
                    else:
                        # must be issued before PV of the diagonal job
                        # (h=0, r, j=r) at slot base + r + 5
                        pos = min(4 + 2 * c, r + 4)
                    parts_at.setdefault(base + pos, []).append(vp)
            outp_after = {3: 0, 11: 1, 23: 2, 39: 3}
            for k in range(n + 5):
                for part in parts_at.get(k, ()):
                    part()
                if k < n:
                    stage_a(jobs[k])
                if 0 <= k - 5 < n:
                    stage_b2(jobs[k - 5])
                if 0 <= k - 2 < n:
                    stage_b1a(jobs[k - 2])
                if 0 <= k - 3 < n:
                    stage_b1b(jobs[k - 3])
                if (k - 5) in outp_after:
                    out_proj(outp_after[k - 5])

    nc.compile()
    return nc


def make_in_maps(x, Wq_w, Wq_b, Wk_w, Wk_b, Wv_w, Wv_b, Wo_w, Wo_b):
    from ml_dtypes import bfloat16

    def bfc(a):
        return np.ascontiguousarray(np.asarray(a, np.float32).astype(bfloat16))

    x = np.asarray(x, np.float32)
    Wq8 = np.asarray(Wq_w, np.float32) / 8.0
    bq8 = np.asarray(Wq_b, np.float32) / 8.0
    wqT = bfc(Wq8.T)
    wkT = bfc(np.asarray(Wk_w, np.float32).T)
    wvT = bfc(np.asarray(Wv_w, np.float32).T)
    woT = bfc(np.asarray(Wo_w, np.float32).T)

    def w_lay(wt, dsl):      # [D, DCORE slice] -> [P, 8, DCORE]
        return np.ascontiguousarray(
            wt[:, dsl].reshape(8, P, DCORE).transpose(1, 0, 2))

    def b_lay(bv):           # [DCORE] -> [P, 2]
        return np.ascontiguousarray(np.asarray(bv, np.float32)
                                    .reshape(2, P).T)

    # additive mask for the in-chunk diagonal: 0 where k<=q else -1e30
    tri = (np.arange(P)[:, None] <= np.arange(P)[None, :])
    maskb = np.where(tri, 0.0, -1e30).astype(np.float32)

    xTb = [bfc(x[b].T) for b in range(B)]
    xh = [np.ascontiguousarray(
        t.reshape(8, P, NB, WIN).transpose(1, 2, 0, 3)) for t in xTb]

    in_maps = []
    for core in range(NCORES):
        b = core // 4
        h0 = (core % 4) * HPC
        dsl = slice(h0 * DK, (h0 + HPC) * DK)
        bv_core = np.asarray(Wv_b, np.float32)[dsl]
        # suffix colsum(v) table computed on host from the rounded operands:
        # colsum_j(v) = (sum_{s in block j} x[s,:]) @ WvT_core + 512*bv
        wvT_core = np.ascontiguousarray(wvT[:, dsl]).astype(np.float32)
        xb32 = xTb[b].astype(np.float32)
        rowsum = np.stack([xb32[:, j * WIN:(j + 1) * WIN].sum(axis=1)
                           for j in range(NB)])            # [NB, D]
        cs = rowsum @ wvT_core + WIN * bv_core[None, :]     # [NB, DCORE]
        sfx_full = np.zeros((NB, DCORE), np.float32)
        for bi in range(NB - 1):
            sfx_full[bi] = cs[bi + 1:].sum(axis=0)
        sfx = np.zeros((DK, 2, 2, NB), np.float32)
        for hc in range(2):
            for half in range(2):
                for bi in range(NB):
                    sfx[:, half, hc, bi] = sfx_full[bi][
                        hc * P + half * DK: hc * P + half * DK + DK]
        in_maps.append({
            "xT": xh[b],
            "wqT": w_lay(wqT, dsl),
            "wkT": w_lay(wkT, dsl),
            "wvT": w_lay(wvT, dsl),
            "woT": np.ascontiguousarray(
                woT[dsl, :].reshape(2, P, D).transpose(1, 0, 2)),
            "bq": b_lay(bq8[dsl]),
            "bk": b_lay(np.asarray(Wk_b, np.float32)[dsl]),
            "bvr": np.ascontiguousarray(np.broadcast_to(bv_core, (P, DCORE))),
            "maskd": maskb,
            "sfxd": sfx,
        })
    return in_maps


def kernel(**inputs):
    if "nc" not in _CACHE:
        _CACHE["nc"] = build_nc()
    nc = _CACHE["nc"]
    in_maps = make_in_maps(**inputs)
    kw = {}
    if TRACE:
        kw["trace"] = True
        if TRACE_CORES is not None:
            kw["trace_cores"] = TRACE_CORES
    res = run_bass_kernel_spmd(nc, in_maps, list(range(NCORES)), **kw)
    _CACHE["last_result"] = res

    bo = np.asarray(inputs["Wo_b"], np.float32)
    out = np.zeros((B, S, D), np.float32)
    for b in range(B):
        acc = np.zeros((D, S), np.float32)
        for core in range(b * 4, b * 4 + 4):
            acc += np.asarray(res.results[core]["outT"], np.float32)
        out[b] = acc.T + bo
    return out
